# revision 108
# baseline (speedup 1.0000x reference)
"""Trainium2 Bass kernel for nn_Attention (Luong 'general' attention with
log-softmax scores used directly as mixing weights).

Math (per batch b):
    dw     = dec @ W                      [T, D]
    affine = dw @ enc^T                   [T, S]
    score  = log_softmax(affine, -1)      [T, S]   (output)
    ctx    = score @ enc                  [T, D]
    out    = [ctx, dec] @ proj_w + proj_b [T, D]   (output)

Key reassociation (exact algebra): score = affine - lse (lse = logsumexp rows)
    ctx = affine@enc - lse x colsum(enc) = dw @ (enc^T enc) - lse x cs
so the score bmm becomes dw @ G with G = enc^T enc (Gram, symmetric) and we
never need score/affine transposed. Then:
    out = dw @ (G @ pw1) + dec @ pw2 + (-lse) x (cs @ pw1) + 1 x b
The rank-1 terms are fused into the PSUM accumulation as one K=2 matmul.

Sharding: pure data-parallel over batch; B=8 batches -> 8 NeuronCores,
one batch per core. Weights replicated. No collectives.

All matmuls run in fp32r (fp32 storage, 12-bit-mantissa multiply, fp32
accumulate) for 4x PE throughput vs fp32.
"""

from contextlib import ExitStack

import numpy as np

import concourse.bacc as bacc
import concourse.mybir as mybir
import concourse.tile as tile
from concourse.bass import ds, ts
from concourse.bass_utils import run_bass_kernel_spmd
from concourse.masks import make_identity

F32 = mybir.dt.float32
F32R = mybir.dt.float32r
I32 = mybir.dt.int32
AX = mybir.AxisListType.X
EXP = mybir.ActivationFunctionType.Exp
ALU = mybir.AluOpType
P = 128

# minimax-ish poly for ln(1+t) on t in [0,1) (abs err ~2e-4; far below the
# fp32r noise floor of this kernel). Derived from a least-squares fit on
# Chebyshev nodes, constant term pinned to 0.
_LN_POLY = [
    -0.017872292608753292, 0.08418346872719132, -0.19222341699173204,
    0.31687103935652705, -0.49770163196183786, 0.9998888323941338,
]  # degree 6..1 coefficients (Horner from highest), times t at the end
_LN2 = 0.6931471805599453


def _dve_ln(nc, smx, lns, s):
    """lns = ln(s) computed on DVE only (frexp + degree-6 poly).

    Avoids the ACT Ln activation, whose function-set load serializes with
    Exp every softmax tile. s must be >= 1 (true for sum of exp(x - max)).
    """
    bits = s.bitcast(I32)
    e_f = smx.tile([P, 1], F32, tag="ln_ef")
    e_i = smx.tile([P, 1], I32, tag="ln_ei")
    # raw biased exponent; the -127 bias is folded into the poly tail below
    nc.vector.tensor_scalar(e_i[:], bits, 23, None, ALU.logical_shift_right)
    nc.vector.tensor_copy(e_f[:], e_i[:])  # int32 -> f32 cast
    m_i = smx.tile([P, 1], I32, tag="ln_mi")
    nc.vector.tensor_scalar(m_i[:], bits, 0x007FFFFF, 0x3F800000,
                            ALU.bitwise_and, ALU.bitwise_or)
    t = smx.tile([P, 1], F32, tag="ln_t")
    nc.vector.tensor_scalar(t[:], m_i[:].bitcast(F32), 1.0, None, ALU.subtract)
    p = smx.tile([P, 1], F32, tag="ln_p")
    nc.vector.memset(p[:], _LN_POLY[0])
    for c in _LN_POLY[1:]:
        nc.vector.tensor_scalar(p[:], p[:], t[:], c, ALU.mult, ALU.add)
    # p = p*t - 127*ln2 ; lns = e_f*ln2 + p
    nc.vector.tensor_scalar(p[:], p[:], t[:], -127.0 * _LN2, ALU.mult, ALU.add)
    nc.vector.tensor_scalar(lns[:], e_f[:], _LN2, p[:], ALU.mult, ALU.add)


def build_program(S, T, D, num_devices=8):
    """Build the per-core Bass program. Same program runs on every core."""
    DO = D // P          # feature-dim k-tiles
    SO = S // P          # enc seq partition tiles
    TO = T // P          # dec seq partition tiles
    CHS = min(512, S)    # free-dim chunk for S
    NCHS = S // CHS
    CHD = min(512, D)    # free-dim chunk for D
    NCHD = D // CHD
    TQ = min(512, T)     # t rows per quarter-block
    NQ = T // TQ
    TPQ = TQ // P        # t-tiles per quarter

    nc = bacc.Bacc("TRN2", debug=False, num_devices=num_devices)

    enc_d = nc.dram_tensor("enc", [S, D], F32, kind="ExternalInput").ap()
    dec_d = nc.dram_tensor("dec", [T, D], F32, kind="ExternalInput").ap()
    w_d = nc.dram_tensor("weight", [D, D], F32, kind="ExternalInput").ap()
    pw_d = nc.dram_tensor("proj_w", [2 * D, D], F32, kind="ExternalInput").ap()
    pb_d = nc.dram_tensor("proj_b", [D], F32, kind="ExternalInput").ap()
    out_d = nc.dram_tensor("out", [T, D], F32, kind="ExternalOutput").ap()
    score_d = nc.dram_tensor("score", [T, S], F32, kind="ExternalOutput").ap()

    with ExitStack() as ctx:
        tc = ctx.enter_context(tile.TileContext(nc, pool_alloc_mode="queue"))

        # ---- persistent pools (whole kernel) ----
        pers = ctx.enter_context(tc.tile_pool(name="pers", bufs=1))
        psT = ctx.enter_context(tc.tile_pool(name="psT", bufs=2, space="PSUM"))
        psM = ctx.enter_context(tc.tile_pool(name="psM", bufs=2, space="PSUM"))

        identity = pers.tile([P, P], F32, tag="identity")
        make_identity(nc, identity[:])
        # f32r identity for transposing already-rounded tensors (1.5 cyc/row
        # vs 2.0 for f32 transpose-mode)
        identity_r = pers.tile([P, P], F32R, tag="identity_r")
        nc.vector.tensor_copy(identity_r[:], identity[:])
        cb = pers.tile([2, D], F32R, tag="cb")          # row0=-csp, row1=proj_b
        # colsum(enc) as columns, d on partitions; stored in duplicated pairs
        # (cols 2*dh and 2*dh+1 identical) because fp32r matmuls need even
        # free sizes on every operand.
        cs_col = pers.tile([P, 2 * DO], F32R, tag="cs_col")
        lse_all = pers.tile([P, P], F32, tag="lse_all")  # col tt = lse of t-tile tt
        nc.vector.memset(lse_all[:], 0.0)


        # H = W @ (G @ pw1) + pw2  [d_lo, d_hi, d'] — folds the whole
        # ctx-and-pw2 projection into one [D, D] operand so phase E needs a
        # single 8-deep k-loop per chunk instead of 17.
        h_pool = ctx.enter_context(tc.tile_pool(name="h", bufs=1))
        H = h_pool.tile([P, DO, D], F32R, tag="H")
        # encT lives through phase D only; closed before phase E so its 64KB
        # can be reused there (manual stack keeps pool release LIFO)
        encT_stack = ExitStack()
        encT_pool = encT_stack.enter_context(tc.tile_pool(name="encT", bufs=1))
        encT = encT_pool.tile([P, DO, S], F32R, tag="encT")   # [e_lo, e_hi, s]

        # =========== Phase A: enc load/round, encT, G, cs ===========
        with tc.tile_pool(name="g", bufs=1) as g_pool:
            G = g_pool.tile([P, DO, D], F32R, tag="G")        # [e_lo, e_hi, d]
            with tc.tile_pool(name="encr", bufs=1) as encr_pool, \
                 tc.tile_pool(name="stgA", bufs=4) as stgA, \
                 tc.tile_pool(name="smA", bufs=2) as smA:
                enc_r = encr_pool.tile([P, SO, D], F32R, tag="enc_r")
                for so in range(SO):
                    for gh in range(NCHD):
                        stg = stgA.tile([P, CHD], F32, tag="stg")
                        nc.sync.dma_start(
                            stg[:], enc_d[ts(so, P), ds(gh * CHD, CHD)]
                        )
                        nc.vector.tensor_copy(
                            enc_r[:, so, ds(gh * CHD, CHD)], stg[:]
                        )
                # encT via PE transposes of 128x128 blocks, batched 4 per
                # PSUM tile so one strided copyback covers 4 transposes
                for so in range(SO):
                    for dh0 in range(0, DO, 4):
                        nb = min(4, DO - dh0)
                        pst = psT.tile([P, 4 * P], F32R, tag="tr4")
                        for i in range(nb):
                            nc.tensor.transpose(
                                pst[:, ts(i, P)],
                                enc_r[:, so, ts(dh0 + i, P)],
                                identity_r[:],
                            )
                        nc.scalar.copy(
                            encT[:, dh0 : dh0 + nb, ts(so, P)],
                            pst[:, 0 : nb * P].rearrange("p (b x) -> p b x", x=P),
                        )
                # G = enc^T @ enc (symmetric): compute blocks on/below the
                # diagonal, mirror the rest via PE transposes of computed ones
                for eh in range(DO):
                    for gh in range(NCHD):
                        if eh * P < gh * CHD:
                            continue
                        ps = psM.tile([P, CHD], F32, tag="mm")
                        for so in range(SO):
                            nc.tensor.matmul(
                                ps[:],
                                enc_r[:, so, ts(eh, P)],
                                enc_r[:, so, ds(gh * CHD, CHD)],
                                start=(so == 0),
                                stop=(so == SO - 1),
                            )
                        nc.vector.tensor_copy(G[:, eh, ds(gh * CHD, CHD)], ps[:])
                for eh in range(DO):
                    for gh in range(NCHD):
                        if eh * P >= gh * CHD:
                            continue
                        for j0 in range(0, CHD // P, 4):
                            nb = min(4, CHD // P - j0)
                            pst = psT.tile([P, 4 * P], F32R, tag="tr4")
                            for i in range(nb):
                                src_h = (gh * CHD) // P + j0 + i
                                nc.tensor.transpose(
                                    pst[:, ts(i, P)],
                                    G[:, src_h, ts(eh, P)],
                                    identity_r[:],
                                )
                            nc.vector.tensor_copy(
                                G[:, eh, ds(gh * CHD + j0 * P, nb * P)],
                                pst[:, 0 : nb * P],
                            )
                # cs_col[d] = sum_s enc[s, d] = row-sums of encT (free-dim
                # reduce on DVE; keeps PE free)
                for dh in range(DO):
                    cst = smA.tile([P, 1], F32, tag="cst")
                    nc.vector.reduce_sum(cst[:], encT[:, dh, :].bitcast(F32), axis=AX)
                    nc.vector.tensor_copy(
                        cs_col[:, 2 * dh : 2 * dh + 2],
                        cst[:, 0:1].to_broadcast((P, 2)),
                    )

            # =========== Phase B: Gp = G @ pw1, cb = [-csp; b] ===========
            with tc.tile_pool(name="gp", bufs=1) as gp_pool:
                Gp = gp_pool.tile([P, DO, D], F32R, tag="Gp")  # [e_lo, e_hi, d']
                with tc.tile_pool(name="pw1", bufs=1) as pw1_pool, \
                     tc.tile_pool(name="stgB", bufs=3) as stgB, \
                     tc.tile_pool(name="cbp", bufs=1) as cbp:
                    cbstg = cbp.tile([2, D], F32, tag="cbstg")
                    # pw1 streamed per 512-col half; Gp/csp looped gh-outer
                    for gh in range(NCHD):
                        pw1h = pw1_pool.tile([P, DO, CHD], F32R, tag="pw1h")
                        for dh in range(DO):
                            stg = stgB.tile([P, CHD], F32, tag="stg")
                            nc.sync.dma_start(
                                stg[:], pw_d[ts(dh, P), ds(gh * CHD, CHD)]
                            )
                            nc.vector.tensor_copy(pw1h[:, dh, :], stg[:])
                        # Gp[e, d'] = sum_d G[e, d] pw1[d, d']; lhsT = G (symmetry)
                        for eh in range(DO):
                            ps = psM.tile([P, CHD], F32, tag="mm")
                            for dh in range(DO):
                                nc.tensor.matmul(
                                    ps[:],
                                    G[:, dh, ts(eh, P)],
                                    pw1h[:, dh, :],
                                    start=(dh == 0),
                                    stop=(dh == DO - 1),
                                )
                            nc.vector.tensor_copy(Gp[:, eh, ds(gh * CHD, CHD)], ps[:])
                        # csp = cs @ pw1 (row);  cb row0 = -csp
                        ps = psM.tile([2, CHD], F32, tag="mm")
                        for dh in range(DO):
                            nc.tensor.matmul(
                                ps[:],
                                cs_col[:, 2 * dh : 2 * dh + 2],
                                pw1h[:, dh, :],
                                start=(dh == 0),
                                stop=(dh == DO - 1),
                            )
                        nc.scalar.mul(cbstg[0:1, ds(gh * CHD, CHD)], ps[0:1, :], -1.0)
                    # cb row1 = proj_b (DMA can write partition 1; DVE cannot)
                    nc.sync.dma_start(cbstg[1:2, :], pb_d[None, :])
                    nc.vector.tensor_copy(cb[:], cbstg[:])

                # ======= Phase B2: WT into G's (now dead) tile, H = WT.T@Gp + pw2
                WT = G  # reuse: same shape/dtype, G fully consumed above
                with tc.tile_pool(name="stgW", bufs=3) as stgW:
                    for dh in range(DO):
                        for half in range(0, DO, 4):
                            nb = min(4, DO - half)
                            stg = stgW.tile([P, 4 * P], F32, tag="wstg")
                            nc.sync.dma_start(
                                stg[:, 0 : nb * P],
                                w_d[ts(dh, P), ds(half * P, nb * P)],
                            )
                            pst = psT.tile([P, 4 * P], F32, tag="tr4")
                            for i in range(nb):
                                nc.tensor.transpose(
                                    pst[:, ts(i, P)], stg[:, ts(i, P)], identity[:]
                                )
                            nc.vector.tensor_copy(
                                WT[:, half : half + nb, ts(dh, P)],
                                pst[:, 0 : nb * P].rearrange("p (b x) -> p b x", x=P),
                            )
                    # H[d, d'] = sum_e W[d, e] Gp[e, d'] + pw2[d, d']
                    for dh in range(DO):
                        for gh in range(NCHD):
                            ps = psM.tile([P, CHD], F32, tag="mm")
                            for eh in range(DO):
                                nc.tensor.matmul(
                                    ps[:],
                                    WT[:, eh, ts(dh, P)],
                                    Gp[:, eh, ds(gh * CHD, CHD)],
                                    start=(eh == 0),
                                    stop=(eh == DO - 1),
                                )
                            stg2 = stgW.tile([P, CHD], F32, tag="pw2stg")
                            nc.sync.dma_start(
                                stg2[:], pw_d[ts(DO + dh, P), ds(gh * CHD, CHD)]
                            )
                            nc.vector.tensor_tensor(
                                H[:, dh, ds(gh * CHD, CHD)], ps[:], stg2[:], ALU.add
                            )

        # =========== Phase C: load W (natural [d, e] layout) ===========
        with tc.tile_pool(name="w", bufs=1) as w_pool:
            with tc.tile_pool(name="stgC", bufs=2) as stgC:
                W_r = w_pool.tile([P, DO, D], F32R, tag="W")
                for dh in range(DO):
                    stg = stgC.tile([P, D], F32, tag="stg")
                    nc.sync.dma_start(stg[:], w_d[ts(dh, P), :])
                    nc.vector.tensor_copy(W_r[:, dh, :], stg[:])

            # =========== Phase D: per quarter: decT, dwT, affine, softmax, score ===========
            # DRAM scratch for decT (reloaded in phase E)
            dram = ctx.enter_context(tc.tile_pool(name="dram", bufs=1, space="DRAM"))
            decT_dram = dram.tile([P, DO, T], F32, tag="decT_dram")

            with tc.tile_pool(name="dstg", bufs=4) as dstg, \
                 tc.tile_pool(name="dect", bufs=1) as dect_pool, \
                 tc.tile_pool(name="dwt", bufs=1) as dwt_pool, \
                 tc.tile_pool(name="aff", bufs=2) as aff_pool, \
                 tc.tile_pool(name="expp", bufs=2) as exp_pool, \
                 tc.tile_pool(name="smx", bufs=4) as smx, \
                 tc.tile_pool(name="psA", bufs=4, space="PSUM") as psA:
                for q in range(NQ):
                    decTq = dect_pool.tile([P, DO, TQ], F32R, tag="decTq")
                    dwTq = dwt_pool.tile([P, DO, TQ], F32R, tag="dwTq")
                    for j2 in range(TPQ):
                        t0 = q * TQ + j2 * P
                        stg = dstg.tile([P, D], F32, tag="dstg")
                        nc.sync.dma_start(stg[:], dec_d[ds(t0, P), :])
                        # transpose the raw f32; the f32r round happens in the
                        # PSUM->SBUF copyback (decTq is f32r)
                        for dh0 in range(0, DO, 4):
                            nb = min(4, DO - dh0)
                            pst = psT.tile([P, 4 * P], F32, tag="tr4")
                            for i in range(nb):
                                nc.tensor.transpose(
                                    pst[:, ts(i, P)],
                                    stg[:, ts(dh0 + i, P)],
                                    identity[:],
                                )
                            nc.vector.tensor_copy(
                                decTq[:, dh0 : dh0 + nb, ds(j2 * P, P)],
                                pst[:, 0 : nb * P].rearrange("p (b x) -> p b x", x=P),
                            )
                    nc.scalar.dma_start(
                        decT_dram[:, :, ds(q * TQ, TQ)], decTq[:].bitcast(F32)
                    )

                    # dwT[e, t] = sum_d W[d, e] decT[d, t]
                    for eh in range(DO):
                        ps = psM.tile([P, TQ], F32, tag="mm")
                        for dh in range(DO):
                            nc.tensor.matmul(
                                ps[:],
                                W_r[:, dh, ts(eh, P)],
                                decTq[:, dh, :],
                                start=(dh == 0),
                                stop=(dh == DO - 1),
                            )
                        nc.vector.tensor_copy(dwTq[:, eh, :], ps[:])
                    # affine + log-softmax + score per t-tile
                    for j2 in range(TPQ):
                        tt = q * TPQ + j2
                        aff = aff_pool.tile([P, S], F32, tag="aff")
                        mxp = smx.tile([P, NCHS], F32, tag="mxp")
                        smp = smx.tile([P, NCHS], F32, tag="smp")
                        m = smx.tile([P, 1], F32, tag="m")
                        negm = smx.tile([P, 1], F32, tag="negm")
                        ssum = smx.tile([P, 1], F32, tag="ssum")
                        lns = smx.tile([P, 1], F32, tag="lns")
                        lse = smx.tile([P, 1], F32, tag="lse")
                        for ch in range(NCHS):
                            ps = psA.tile([P, CHS], F32, tag="aff")
                            for eh in range(DO):
                                nc.tensor.matmul(
                                    ps[:],
                                    dwTq[:, eh, ds(j2 * P, P)],
                                    encT[:, eh, ds(ch * CHS, CHS)],
                                    start=(eh == 0),
                                    stop=(eh == DO - 1),
                                )
                            nc.vector.tensor_copy(aff[:, ds(ch * CHS, CHS)], ps[:])
                            nc.vector.reduce_max(
                                mxp[:, ch : ch + 1], aff[:, ds(ch * CHS, CHS)], axis=AX
                            )
                        nc.vector.reduce_max(m[:], mxp[:], axis=AX)
                        nc.vector.tensor_scalar_mul(negm[:], m[:], -1.0)
                        for ch in range(NCHS):
                            ex = exp_pool.tile([P, CHS], F32, tag="ex")
                            nc.scalar.activation(
                                ex[:],
                                aff[:, ds(ch * CHS, CHS)],
                                EXP,
                                bias=negm[:],
                                accum_out=smp[:, ch : ch + 1],
                            )
                        nc.vector.reduce_sum(ssum[:], smp[:], axis=AX)
                        _dve_ln(nc, smx, lns, ssum[:])
                        nc.vector.tensor_add(lse[:], m[:], lns[:])
                        nc.vector.tensor_copy(lse_all[:, tt : tt + 1], lse[:])
                        for ch in range(NCHS):
                            nc.vector.tensor_scalar(
                                aff[:, ds(ch * CHS, CHS)],
                                aff[:, ds(ch * CHS, CHS)],
                                lse[:],
                                None,
                                op0=mybir.AluOpType.subtract,
                            )
                        # output stream rides the ACT HWDGE queues so it never
                        # queues behind the input loads on SP
                        nc.scalar.dma_start(score_d[ds(tt * P, P), :], aff[:])

        # =========== Phase E: out = decT.T @ H + rank1([-lse;1] x [csp;b]) ===========
        encT_stack.close()
        with tc.tile_pool(name="lbp", bufs=1) as lbp, \
             tc.tile_pool(name="slc", bufs=3) as slc, \
             tc.tile_pool(name="slcr", bufs=3) as slcr, \
             tc.tile_pool(name="outp", bufs=2) as outp:
            # lb = [lse_row; ones_row] as [2, T] fp32r
            lb = lbp.tile([2, T], F32R, tag="lb")
            pst = psT.tile([P, 4 * P], F32, tag="tr4")
            nc.tensor.transpose(pst[:, 0:P], lse_all[:], identity[:])
            LtT = lbp.tile([TO, P], F32, tag="LtT")
            nc.vector.tensor_copy(LtT[:], pst[0:TO, 0:P])
            lrow2 = lbp.tile([2, T], F32, tag="lrow2")
            nc.vector.memset(lrow2[:], 1.0)  # row 1 stays all-ones
            for j in range(TO):
                nc.sync.dma_start(lrow2[0:1, ts(j, P)], LtT[j : j + 1, :])
            nc.vector.tensor_copy(lb[:], lrow2[:])
            # out sweep
            for tt in range(TO):
                dsl = slc.tile([P, DO, P], F32, tag="dsl")
                nc.sync.dma_start(dsl[:], decT_dram[:, :, ts(tt, P)])
                dslr = slcr.tile([P, DO, P], F32R, tag="dslr")
                nc.scalar.copy(dslr[:], dsl[:])
                outsb = outp.tile([P, D], F32, tag="outsb")
                for oh in range(NCHD):
                    ps = psM.tile([P, CHD], F32, tag="mm")
                    for dh in range(DO):
                        nc.tensor.matmul(
                            ps[:],
                            dslr[:, dh, :],
                            H[:, dh, ds(oh * CHD, CHD)],
                            start=(dh == 0),
                            stop=False,
                        )
                    nc.tensor.matmul(
                        ps[:],
                        lb[:, ts(tt, P)],
                        cb[:, ds(oh * CHD, CHD)],
                        start=False,
                        stop=True,
                        skip_group_check=True,
                    )
                    nc.scalar.copy(outsb[:, ds(oh * CHD, CHD)], ps[:])
                nc.scalar.dma_start(out_d[ts(tt, P), :], outsb[:])

    nc.compile()
    return nc


_NC_CACHE = {}


def _get_program(S, T, D, num_devices):
    key = (S, T, D, num_devices)
    if key not in _NC_CACHE:
        _NC_CACHE[key] = build_program(S, T, D, num_devices=num_devices)
    return _NC_CACHE[key]


def run(inputs, trace=False):
    """Run the kernel on 8 cores; returns (out, score[, BassKernelResults])."""
    enc = np.ascontiguousarray(np.asarray(inputs["enc"], dtype=np.float32))
    dec = np.ascontiguousarray(np.asarray(inputs["dec"], dtype=np.float32))
    w = np.ascontiguousarray(np.asarray(inputs["weight"], dtype=np.float32))
    pw = np.ascontiguousarray(np.asarray(inputs["proj_w"], dtype=np.float32))
    pb = np.ascontiguousarray(np.asarray(inputs["proj_b"], dtype=np.float32))
    B, S, D = enc.shape
    T = dec.shape[1]
    nc = _get_program(S, T, D, num_devices=B)
    in_maps = [
        {
            "enc": np.ascontiguousarray(enc[c]),
            "dec": np.ascontiguousarray(dec[c]),
            "weight": w,
            "proj_w": pw,
            "proj_b": pb,
        }
        for c in range(B)
    ]
    res = run_bass_kernel_spmd(nc, in_maps, core_ids=list(range(B)), trace=trace)
    out = np.stack([res.results[c]["out"] for c in range(B)])
    score = np.stack([res.results[c]["score"] for c in range(B)])
    if trace:
        return out, score, res
    return out, score


def kernel(**inputs):
    out, score = run(inputs, trace=False)
    return out, score


# revision 123
# speedup vs baseline: 1.0041x; 1.0041x over previous
"""Trainium2 Bass kernel for nn_Attention (Luong 'general' attention with
log-softmax scores used directly as mixing weights).

Math (per batch b):
    dw     = dec @ W                      [T, D]
    affine = dw @ enc^T                   [T, S]
    score  = log_softmax(affine, -1)      [T, S]   (output)
    ctx    = score @ enc                  [T, D]
    out    = [ctx, dec] @ proj_w + proj_b [T, D]   (output)

Key reassociation (exact algebra): score = affine - lse (lse = logsumexp rows)
    ctx = affine@enc - lse x colsum(enc) = dw @ (enc^T enc) - lse x cs
so the score bmm becomes dw @ G with G = enc^T enc (Gram, symmetric) and we
never need score/affine transposed. Then:
    out = dw @ (G @ pw1) + dec @ pw2 + (-lse) x (cs @ pw1) + 1 x b
The rank-1 terms are fused into the PSUM accumulation as one K=2 matmul.

Sharding: pure data-parallel over batch; B=8 batches -> 8 NeuronCores,
one batch per core. Weights replicated. No collectives.

All matmuls run in fp32r (fp32 storage, 12-bit-mantissa multiply, fp32
accumulate) for 4x PE throughput vs fp32.
"""

from contextlib import ExitStack

import numpy as np

import concourse.bacc as bacc
import concourse.mybir as mybir
import concourse.tile as tile
from concourse.bass import ds, ts
from concourse.bass_utils import run_bass_kernel_spmd
from concourse.masks import make_identity

F32 = mybir.dt.float32
F32R = mybir.dt.float32r
I32 = mybir.dt.int32
AX = mybir.AxisListType.X
EXP = mybir.ActivationFunctionType.Exp
ALU = mybir.AluOpType
P = 128

# minimax-ish poly for ln(1+t) on t in [0,1) (abs err ~2e-4; far below the
# fp32r noise floor of this kernel). Derived from a least-squares fit on
# Chebyshev nodes, constant term pinned to 0.
_LN_POLY = [
    -0.017872292608753292, 0.08418346872719132, -0.19222341699173204,
    0.31687103935652705, -0.49770163196183786, 0.9998888323941338,
]  # degree 6..1 coefficients (Horner from highest), times t at the end
_LN2 = 0.6931471805599453


def _dve_ln(nc, smx, lns, s):
    """lns = ln(s) computed on DVE only (frexp + degree-6 poly).

    Avoids the ACT Ln activation, whose function-set load serializes with
    Exp every softmax tile. s must be >= 1 (true for sum of exp(x - max)).
    """
    bits = s.bitcast(I32)
    e_f = smx.tile([P, 1], F32, tag="ln_ef")
    e_i = smx.tile([P, 1], I32, tag="ln_ei")
    # raw biased exponent; the -127 bias is folded into the poly tail below
    nc.vector.tensor_scalar(e_i[:], bits, 23, None, ALU.logical_shift_right)
    nc.vector.tensor_copy(e_f[:], e_i[:])  # int32 -> f32 cast
    m_i = smx.tile([P, 1], I32, tag="ln_mi")
    nc.vector.tensor_scalar(m_i[:], bits, 0x007FFFFF, 0x3F800000,
                            ALU.bitwise_and, ALU.bitwise_or)
    t = smx.tile([P, 1], F32, tag="ln_t")
    nc.vector.tensor_scalar(t[:], m_i[:].bitcast(F32), 1.0, None, ALU.subtract)
    p = smx.tile([P, 1], F32, tag="ln_p")
    nc.vector.memset(p[:], _LN_POLY[0])
    for c in _LN_POLY[1:]:
        nc.vector.tensor_scalar(p[:], p[:], t[:], c, ALU.mult, ALU.add)
    # p = p*t - 127*ln2 ; lns = e_f*ln2 + p
    nc.vector.tensor_scalar(p[:], p[:], t[:], -127.0 * _LN2, ALU.mult, ALU.add)
    nc.vector.tensor_scalar(lns[:], e_f[:], _LN2, p[:], ALU.mult, ALU.add)


def build_program(S, T, D, num_devices=8):
    """Build the per-core Bass program. Same program runs on every core."""
    DO = D // P          # feature-dim k-tiles
    SO = S // P          # enc seq partition tiles
    TO = T // P          # dec seq partition tiles
    CHS = min(512, S)    # free-dim chunk for S
    NCHS = S // CHS
    CHD = min(512, D)    # free-dim chunk for D
    NCHD = D // CHD
    TQ = min(512, T)     # t rows per quarter-block
    NQ = T // TQ
    TPQ = TQ // P        # t-tiles per quarter

    nc = bacc.Bacc("TRN2", debug=False, num_devices=num_devices)

    enc_d = nc.dram_tensor("enc", [S, D], F32, kind="ExternalInput").ap()
    dec_d = nc.dram_tensor("dec", [T, D], F32, kind="ExternalInput").ap()
    w_d = nc.dram_tensor("weight", [D, D], F32, kind="ExternalInput").ap()
    pw_d = nc.dram_tensor("proj_w", [2 * D, D], F32, kind="ExternalInput").ap()
    pb_d = nc.dram_tensor("proj_b", [D], F32, kind="ExternalInput").ap()
    out_d = nc.dram_tensor("out", [T, D], F32, kind="ExternalOutput").ap()
    score_d = nc.dram_tensor("score", [T, S], F32, kind="ExternalOutput").ap()

    with ExitStack() as ctx:
        tc = ctx.enter_context(tile.TileContext(nc, pool_alloc_mode="queue"))

        # ---- persistent pools (whole kernel) ----
        pers = ctx.enter_context(tc.tile_pool(name="pers", bufs=1))
        psT = ctx.enter_context(tc.tile_pool(name="psT", bufs=2, space="PSUM"))
        psM = ctx.enter_context(tc.tile_pool(name="psM", bufs=2, space="PSUM"))

        identity = pers.tile([P, P], F32, tag="identity")
        make_identity(nc, identity[:])
        # f32r identity for transposing already-rounded tensors (1.5 cyc/row
        # vs 2.0 for f32 transpose-mode)
        identity_r = pers.tile([P, P], F32R, tag="identity_r")
        nc.vector.tensor_copy(identity_r[:], identity[:])
        cb = pers.tile([2, D], F32R, tag="cb")          # row0=-csp, row1=proj_b
        # colsum(enc) as columns, d on partitions; stored in duplicated pairs
        # (cols 2*dh and 2*dh+1 identical) because fp32r matmuls need even
        # free sizes on every operand.
        cs_col = pers.tile([P, 2 * DO], F32R, tag="cs_col")
        lse_all = pers.tile([P, P], F32, tag="lse_all")  # col tt = lse of t-tile tt
        nc.vector.memset(lse_all[:], 0.0)


        # H = W @ (G @ pw1) + pw2  [d_lo, d_hi, d'] — folds the whole
        # ctx-and-pw2 projection into one [D, D] operand so phase E needs a
        # single 8-deep k-loop per chunk instead of 17.
        h_pool = ctx.enter_context(tc.tile_pool(name="h", bufs=1))
        H = h_pool.tile([P, DO, D], F32R, tag="H")
        # encT lives through phase D only; closed before phase E so its 64KB
        # can be reused there (manual stack keeps pool release LIFO)
        encT_stack = ExitStack()
        encT_pool = encT_stack.enter_context(tc.tile_pool(name="encT", bufs=1))
        encT = encT_pool.tile([P, DO, S], F32R, tag="encT")   # [e_lo, e_hi, s]

        # =========== Phase A: enc load/round, encT, G, cs ===========
        with tc.tile_pool(name="g", bufs=1) as g_pool:
            G = g_pool.tile([P, DO, D], F32R, tag="G")        # [e_lo, e_hi, d]
            with tc.tile_pool(name="encr", bufs=1) as encr_pool, \
                 tc.tile_pool(name="stgA", bufs=5) as stgA, \
                 tc.tile_pool(name="smA", bufs=2) as smA:
                enc_r = encr_pool.tile([P, SO, D], F32R, tag="enc_r")
                for so in range(SO):
                    for gh in range(NCHD):
                        stg = stgA.tile([P, CHD], F32, tag="stg")
                        nc.sync.dma_start(
                            stg[:], enc_d[ts(so, P), ds(gh * CHD, CHD)]
                        )
                        nc.vector.tensor_copy(
                            enc_r[:, so, ds(gh * CHD, CHD)], stg[:]
                        )
                # encT via PE transposes of 128x128 blocks, batched 4 per
                # PSUM tile so one strided copyback covers 4 transposes
                for so in range(SO):
                    for dh0 in range(0, DO, 4):
                        nb = min(4, DO - dh0)
                        pst = psT.tile([P, 4 * P], F32R, tag="tr4")
                        for i in range(nb):
                            nc.tensor.transpose(
                                pst[:, ts(i, P)],
                                enc_r[:, so, ts(dh0 + i, P)],
                                identity_r[:],
                            )
                        nc.scalar.copy(
                            encT[:, dh0 : dh0 + nb, ts(so, P)],
                            pst[:, 0 : nb * P].rearrange("p (b x) -> p b x", x=P),
                        )
                # G = enc^T @ enc (symmetric): compute blocks on/below the
                # diagonal, mirror the rest via PE transposes of computed ones
                for eh in range(DO):
                    for gh in range(NCHD):
                        if eh * P < gh * CHD:
                            continue
                        ps = psM.tile([P, CHD], F32, tag="mm")
                        for so in range(SO):
                            nc.tensor.matmul(
                                ps[:],
                                enc_r[:, so, ts(eh, P)],
                                enc_r[:, so, ds(gh * CHD, CHD)],
                                start=(so == 0),
                                stop=(so == SO - 1),
                            )
                        nc.vector.tensor_copy(G[:, eh, ds(gh * CHD, CHD)], ps[:])
                for eh in range(DO):
                    for gh in range(NCHD):
                        if eh * P >= gh * CHD:
                            continue
                        for j0 in range(0, CHD // P, 4):
                            nb = min(4, CHD // P - j0)
                            pst = psT.tile([P, 4 * P], F32R, tag="tr4")
                            for i in range(nb):
                                src_h = (gh * CHD) // P + j0 + i
                                nc.tensor.transpose(
                                    pst[:, ts(i, P)],
                                    G[:, src_h, ts(eh, P)],
                                    identity_r[:],
                                )
                            # ACT copyback: DVE's FIFO is deep with G-block
                            # copybacks here; ACT is idle after encT
                            nc.scalar.copy(
                                G[:, eh, ds(gh * CHD + j0 * P, nb * P)],
                                pst[:, 0 : nb * P],
                            )
                # cs_col[d] = sum_s enc[s, d] = row-sums of encT (free-dim
                # reduce on DVE; keeps PE free)
                for dh in range(DO):
                    cst = smA.tile([P, 1], F32, tag="cst")
                    nc.vector.reduce_sum(cst[:], encT[:, dh, :].bitcast(F32), axis=AX)
                    nc.vector.tensor_copy(
                        cs_col[:, 2 * dh : 2 * dh + 2],
                        cst[:, 0:1].to_broadcast((P, 2)),
                    )

            # =========== Phase B: Gp = G @ pw1, cb = [-csp; b] ===========
            with tc.tile_pool(name="gp", bufs=1) as gp_pool:
                Gp = gp_pool.tile([P, DO, D], F32R, tag="Gp")  # [e_lo, e_hi, d']
                with tc.tile_pool(name="pw1", bufs=1) as pw1_pool, \
                     tc.tile_pool(name="stgB", bufs=3) as stgB, \
                     tc.tile_pool(name="cbp", bufs=1) as cbp:
                    cbstg = cbp.tile([2, D], F32, tag="cbstg")
                    # pw1 streamed per 512-col half; Gp/csp looped gh-outer
                    for gh in range(NCHD):
                        pw1h = pw1_pool.tile([P, DO, CHD], F32R, tag="pw1h")
                        for dh in range(DO):
                            stg = stgB.tile([P, CHD], F32, tag="stg")
                            nc.sync.dma_start(
                                stg[:], pw_d[ts(dh, P), ds(gh * CHD, CHD)]
                            )
                            nc.vector.tensor_copy(pw1h[:, dh, :], stg[:])
                        # Gp[e, d'] = sum_d G[e, d] pw1[d, d']; lhsT = G (symmetry)
                        for eh in range(DO):
                            ps = psM.tile([P, CHD], F32, tag="mm")
                            for dh in range(DO):
                                nc.tensor.matmul(
                                    ps[:],
                                    G[:, dh, ts(eh, P)],
                                    pw1h[:, dh, :],
                                    start=(dh == 0),
                                    stop=(dh == DO - 1),
                                )
                            nc.vector.tensor_copy(Gp[:, eh, ds(gh * CHD, CHD)], ps[:])
                        # csp = cs @ pw1 (row);  cb row0 = -csp
                        ps = psM.tile([2, CHD], F32, tag="mm")
                        for dh in range(DO):
                            nc.tensor.matmul(
                                ps[:],
                                cs_col[:, 2 * dh : 2 * dh + 2],
                                pw1h[:, dh, :],
                                start=(dh == 0),
                                stop=(dh == DO - 1),
                            )
                        nc.scalar.mul(cbstg[0:1, ds(gh * CHD, CHD)], ps[0:1, :], -1.0)
                    # cb row1 = proj_b (DMA can write partition 1; DVE cannot)
                    nc.sync.dma_start(cbstg[1:2, :], pb_d[None, :])
                    nc.vector.tensor_copy(cb[:], cbstg[:])

                # ======= Phase B2: WT into G's (now dead) tile, H = WT.T@Gp + pw2
                WT = G  # reuse: same shape/dtype, G fully consumed above
                with tc.tile_pool(name="stgW", bufs=3) as stgW:
                    for dh in range(DO):
                        for half in range(0, DO, 4):
                            nb = min(4, DO - half)
                            stg = stgW.tile([P, 4 * P], F32, tag="wstg")
                            nc.sync.dma_start(
                                stg[:, 0 : nb * P],
                                w_d[ts(dh, P), ds(half * P, nb * P)],
                            )
                            pst = psT.tile([P, 4 * P], F32, tag="tr4")
                            for i in range(nb):
                                nc.tensor.transpose(
                                    pst[:, ts(i, P)], stg[:, ts(i, P)], identity[:]
                                )
                            nc.vector.tensor_copy(
                                WT[:, half : half + nb, ts(dh, P)],
                                pst[:, 0 : nb * P].rearrange("p (b x) -> p b x", x=P),
                            )
                    # H[d, d'] = sum_e W[d, e] Gp[e, d'] + pw2[d, d']
                    for dh in range(DO):
                        for gh in range(NCHD):
                            ps = psM.tile([P, CHD], F32, tag="mm")
                            for eh in range(DO):
                                nc.tensor.matmul(
                                    ps[:],
                                    WT[:, eh, ts(dh, P)],
                                    Gp[:, eh, ds(gh * CHD, CHD)],
                                    start=(eh == 0),
                                    stop=(eh == DO - 1),
                                )
                            stg2 = stgW.tile([P, CHD], F32, tag="pw2stg")
                            nc.sync.dma_start(
                                stg2[:], pw_d[ts(DO + dh, P), ds(gh * CHD, CHD)]
                            )
                            nc.vector.tensor_tensor(
                                H[:, dh, ds(gh * CHD, CHD)], ps[:], stg2[:], ALU.add
                            )

        # =========== Phase C: load W (natural [d, e] layout) ===========
        with tc.tile_pool(name="w", bufs=1) as w_pool:
            with tc.tile_pool(name="stgC", bufs=2) as stgC:
                W_r = w_pool.tile([P, DO, D], F32R, tag="W")
                for dh in range(DO):
                    stg = stgC.tile([P, D], F32, tag="stg")
                    nc.sync.dma_start(stg[:], w_d[ts(dh, P), :])
                    nc.vector.tensor_copy(W_r[:, dh, :], stg[:])

            # =========== Phase D: per quarter: decT, dwT, affine, softmax, score ===========
            # DRAM scratch for decT (reloaded in phase E)
            dram = ctx.enter_context(tc.tile_pool(name="dram", bufs=1, space="DRAM"))
            decT_dram = dram.tile([P, DO, T], F32, tag="decT_dram")

            with tc.tile_pool(name="dstg", bufs=4) as dstg, \
                 tc.tile_pool(name="dect", bufs=1) as dect_pool, \
                 tc.tile_pool(name="dwt", bufs=1) as dwt_pool, \
                 tc.tile_pool(name="aff", bufs=2) as aff_pool, \
                 tc.tile_pool(name="expp", bufs=2) as exp_pool, \
                 tc.tile_pool(name="smx", bufs=4) as smx, \
                 tc.tile_pool(name="psA", bufs=4, space="PSUM") as psA:
                for q in range(NQ):
                    decTq = dect_pool.tile([P, DO, TQ], F32R, tag="decTq")
                    dwTq = dwt_pool.tile([P, DO, TQ], F32R, tag="dwTq")
                    for j2 in range(TPQ):
                        t0 = q * TQ + j2 * P
                        stg = dstg.tile([P, D], F32, tag="dstg")
                        nc.sync.dma_start(stg[:], dec_d[ds(t0, P), :])
                        # transpose the raw f32; the f32r round happens in the
                        # PSUM->SBUF copyback (decTq is f32r)
                        for dh0 in range(0, DO, 4):
                            nb = min(4, DO - dh0)
                            pst = psT.tile([P, 4 * P], F32, tag="tr4")
                            for i in range(nb):
                                nc.tensor.transpose(
                                    pst[:, ts(i, P)],
                                    stg[:, ts(dh0 + i, P)],
                                    identity[:],
                                )
                            nc.vector.tensor_copy(
                                decTq[:, dh0 : dh0 + nb, ds(j2 * P, P)],
                                pst[:, 0 : nb * P].rearrange("p (b x) -> p b x", x=P),
                            )
                    nc.scalar.dma_start(
                        decT_dram[:, :, ds(q * TQ, TQ)], decTq[:].bitcast(F32)
                    )

                    # dwT[e, t] = sum_d W[d, e] decT[d, t]
                    for eh in range(DO):
                        ps = psM.tile([P, TQ], F32, tag="mm")
                        for dh in range(DO):
                            nc.tensor.matmul(
                                ps[:],
                                W_r[:, dh, ts(eh, P)],
                                decTq[:, dh, :],
                                start=(dh == 0),
                                stop=(dh == DO - 1),
                            )
                        nc.vector.tensor_copy(dwTq[:, eh, :], ps[:])
                    # affine + log-softmax + score per t-tile
                    for j2 in range(TPQ):
                        tt = q * TPQ + j2
                        aff = aff_pool.tile([P, S], F32, tag="aff")
                        mxp = smx.tile([P, NCHS], F32, tag="mxp")
                        smp = smx.tile([P, NCHS], F32, tag="smp")
                        m = smx.tile([P, 1], F32, tag="m")
                        negm = smx.tile([P, 1], F32, tag="negm")
                        ssum = smx.tile([P, 1], F32, tag="ssum")
                        lns = smx.tile([P, 1], F32, tag="lns")
                        lse = smx.tile([P, 1], F32, tag="lse")
                        for ch in range(NCHS):
                            ps = psA.tile([P, CHS], F32, tag="aff")
                            for eh in range(DO):
                                nc.tensor.matmul(
                                    ps[:],
                                    dwTq[:, eh, ds(j2 * P, P)],
                                    encT[:, eh, ds(ch * CHS, CHS)],
                                    start=(eh == 0),
                                    stop=(eh == DO - 1),
                                )
                            nc.vector.tensor_copy(aff[:, ds(ch * CHS, CHS)], ps[:])
                            nc.vector.reduce_max(
                                mxp[:, ch : ch + 1], aff[:, ds(ch * CHS, CHS)], axis=AX
                            )
                        nc.vector.reduce_max(m[:], mxp[:], axis=AX)
                        nc.vector.tensor_scalar_mul(negm[:], m[:], -1.0)
                        for ch in range(NCHS):
                            ex = exp_pool.tile([P, CHS], F32, tag="ex")
                            nc.scalar.activation(
                                ex[:],
                                aff[:, ds(ch * CHS, CHS)],
                                EXP,
                                bias=negm[:],
                                accum_out=smp[:, ch : ch + 1],
                            )
                        nc.vector.reduce_sum(ssum[:], smp[:], axis=AX)
                        _dve_ln(nc, smx, lns, ssum[:])
                        nc.vector.tensor_add(lse[:], m[:], lns[:])
                        nc.vector.tensor_copy(lse_all[:, tt : tt + 1], lse[:])
                        for ch in range(NCHS):
                            nc.vector.tensor_scalar(
                                aff[:, ds(ch * CHS, CHS)],
                                aff[:, ds(ch * CHS, CHS)],
                                lse[:],
                                None,
                                op0=mybir.AluOpType.subtract,
                            )
                        # output stream rides the ACT HWDGE queues so it never
                        # queues behind the input loads on SP
                        nc.scalar.dma_start(score_d[ds(tt * P, P), :], aff[:])

        # =========== Phase E: out = decT.T @ H + rank1([-lse;1] x [csp;b]) ===========
        encT_stack.close()
        with tc.tile_pool(name="lbp", bufs=1) as lbp, \
             tc.tile_pool(name="slc", bufs=3) as slc, \
             tc.tile_pool(name="slcr", bufs=3) as slcr, \
             tc.tile_pool(name="outp", bufs=2) as outp:
            # lb = [lse_row; ones_row] as [2, T] fp32r
            lb = lbp.tile([2, T], F32R, tag="lb")
            pst = psT.tile([P, 4 * P], F32, tag="tr4")
            nc.tensor.transpose(pst[:, 0:P], lse_all[:], identity[:])
            LtT = lbp.tile([TO, P], F32, tag="LtT")
            nc.vector.tensor_copy(LtT[:], pst[0:TO, 0:P])
            lrow2 = lbp.tile([2, T], F32, tag="lrow2")
            nc.vector.memset(lrow2[:], 1.0)  # row 1 stays all-ones
            for j in range(TO):
                nc.sync.dma_start(lrow2[0:1, ts(j, P)], LtT[j : j + 1, :])
            nc.vector.tensor_copy(lb[:], lrow2[:])
            # out sweep
            for tt in range(TO):
                dsl = slc.tile([P, DO, P], F32, tag="dsl")
                nc.sync.dma_start(dsl[:], decT_dram[:, :, ts(tt, P)])
                dslr = slcr.tile([P, DO, P], F32R, tag="dslr")
                nc.scalar.copy(dslr[:], dsl[:])
                outsb = outp.tile([P, D], F32, tag="outsb")
                for oh in range(NCHD):
                    ps = psM.tile([P, CHD], F32, tag="mm")
                    for dh in range(DO):
                        nc.tensor.matmul(
                            ps[:],
                            dslr[:, dh, :],
                            H[:, dh, ds(oh * CHD, CHD)],
                            start=(dh == 0),
                            stop=False,
                        )
                    nc.tensor.matmul(
                        ps[:],
                        lb[:, ts(tt, P)],
                        cb[:, ds(oh * CHD, CHD)],
                        start=False,
                        stop=True,
                        skip_group_check=True,
                    )
                    nc.scalar.copy(outsb[:, ds(oh * CHD, CHD)], ps[:])
                nc.scalar.dma_start(out_d[ts(tt, P), :], outsb[:])

    nc.compile()
    return nc


_NC_CACHE = {}


def _get_program(S, T, D, num_devices):
    key = (S, T, D, num_devices)
    if key not in _NC_CACHE:
        _NC_CACHE[key] = build_program(S, T, D, num_devices=num_devices)
    return _NC_CACHE[key]


def run(inputs, trace=False):
    """Run the kernel on 8 cores; returns (out, score[, BassKernelResults])."""
    enc = np.ascontiguousarray(np.asarray(inputs["enc"], dtype=np.float32))
    dec = np.ascontiguousarray(np.asarray(inputs["dec"], dtype=np.float32))
    w = np.ascontiguousarray(np.asarray(inputs["weight"], dtype=np.float32))
    pw = np.ascontiguousarray(np.asarray(inputs["proj_w"], dtype=np.float32))
    pb = np.ascontiguousarray(np.asarray(inputs["proj_b"], dtype=np.float32))
    B, S, D = enc.shape
    T = dec.shape[1]
    nc = _get_program(S, T, D, num_devices=B)
    in_maps = [
        {
            "enc": np.ascontiguousarray(enc[c]),
            "dec": np.ascontiguousarray(dec[c]),
            "weight": w,
            "proj_w": pw,
            "proj_b": pb,
        }
        for c in range(B)
    ]
    res = run_bass_kernel_spmd(nc, in_maps, core_ids=list(range(B)), trace=trace)
    out = np.stack([res.results[c]["out"] for c in range(B)])
    score = np.stack([res.results[c]["score"] for c in range(B)])
    if trace:
        return out, score, res
    return out, score


def kernel(**inputs):
    out, score = run(inputs, trace=False)
    return out, score
